# revision 25
# baseline (speedup 1.0000x reference)
"""v4: single-head causal attention (B=8, T=2048, E=1024, H=64) on 8 trn2
cores, data-parallel over batch.

Pipeline per core (natural-v formulation, no final transpose):
  x f32 --HWDGE chunk DMA--> xn --PE f32r transpose--> psum --cast copy-->
  xT bf16 (group tiles, [j, t] layout)
  qkT[128, T] = [Wq|Wk]^T @ xT  (bf16, packed q rows 0:64 / k rows 64:128)
  v[t, 64]    = xT_chunk^T @ Wv (natural layout, ones col appended -> v1)
  scoresT[s-chunk, t] = kT_j^T @ qT  (bf16, diagonal-trimmed)
  wT = exp(scores/8) on ACT (psum -> sbuf bf16), diag masked by tri
  out_psum[t, 65] += wT_j^T @ v1_j   (col 64 = softmax denominator)
  out = psum[:, 0:64] * 1/psum[:, 64] on DVE -> staged -> DMA out

Emission is software-pipelined: loop g interleaves scores/exp(g),
AV(g-1), transposes(g+1), and projections(g+1) to keep PE dense (DVFS).
"""

import numpy as np

import concourse.bass as bass
import concourse.mybir as mybir
from concourse.tile import TileContext, add_dep_helper
from concourse.masks import make_identity, make_upper_triangular
from concourse.bass_utils import run_bass_kernel_spmd

B, T, E, H = 8, 2048, 1024, 64
NT = T // 128   # 16 t-chunks
NE = E // 128   # 8 e-chunks
NG = 4          # t-groups of 4 chunks / 512 cols
F32 = mybir.dt.float32
F32R = mybir.dt.float32r
BF16 = mybir.dt.bfloat16
SCALE = float(H) ** -0.5


def _split_excess_waits(nc: bass.Bass, cap: int = 1) -> int:
    n_split = 0
    for f in nc.m.functions:
        for bb in f.blocks:
            insts = list(bb.instructions)
            out = []
            dirty = False
            for inst in insts:
                si = inst.sync_info
                waits = list(si.on_wait) if si and si.on_wait else []
                if len(waits) > cap:
                    si.on_wait = waits[:cap]
                    for w in waits[cap:]:
                        nop = mybir.InstNoOp(
                            name=f"I-waitsplit-{n_split}", ins=[], outs=[]
                        )
                        nop.engine = inst.engine
                        nop.sync_info = mybir.SyncInfo(on_wait=[w], on_update=[])
                        out.append(nop)
                        n_split += 1
                    dirty = True
                out.append(inst)
            if dirty:
                bb.instructions = out
    return n_split


def build_nc(split_waits: bool = True) -> bass.Bass:
    nc = bass.Bass()
    x = nc.dram_tensor("x", [T, E], F32R, kind="ExternalInput")
    wq = nc.dram_tensor("Wq", [E, H], F32, kind="ExternalInput")
    wk = nc.dram_tensor("Wk", [E, H], F32, kind="ExternalInput")
    wv = nc.dram_tensor("Wv", [E, H], F32, kind="ExternalInput")
    out = nc.dram_tensor("out", [T, H], F32, kind="ExternalOutput")
    x_ap, out_ap = x.ap(), out.ap()

    with TileContext(nc) as tc:
        with (
            tc.tile_pool(name="const", bufs=1) as cpool,
            tc.tile_pool(name="wts", bufs=1) as wpool,
            tc.tile_pool(name="xn", bufs=6) as xnpool,
            tc.tile_pool(name="xtg", bufs=2) as xtpool,
            tc.tile_pool(name="qkv", bufs=1) as qkvpool,
            tc.tile_pool(name="wT", bufs=16) as wtpool,
            tc.tile_pool(name="fin", bufs=8) as finpool,
            tc.tile_pool(name="pT", bufs=2, space="PSUM") as pT,
            tc.tile_pool(name="pQK", bufs=1, space="PSUM") as pQK,
            tc.tile_pool(name="pV", bufs=1, space="PSUM") as pV,
            tc.tile_pool(name="pS", bufs=3, space="PSUM") as pS,
            tc.tile_pool(name="pAV", bufs=1, space="PSUM") as pAV,
        ):
            # ---- input DMAs first: x chunks alternate both HWDGE rings ----
            xn = []
            xdmas = []
            for i in range(NT):
                t_ = xnpool.tile([128, E], F32R, tag="xn", name=f"xn{i}")
                if i == 0:
                    nc.sync.dma_start(t_[:, 0:512], x_ap[0:128, 0:512])
                    d = nc.sync.dma_start(t_[:, 512:E], x_ap[0:128, 512:E])
                else:
                    d = nc.sync.dma_start(
                        t_[:], x_ap[128 * i : 128 * i + 128, :]
                    )
                if i >= 4:
                    add_dep_helper(
                        d.ins, xdmas[i - 4].ins, sync=True,
                        reason="pace x chunk DMAs for in-order completion",
                    )
                xdmas.append(d)
                xn.append(t_)
            # weights on the scalar (ACT) HWDGE ring
            w32 = {}
            for nm, dram in (("q", wq), ("k", wk), ("v", wv)):
                t32 = wpool.tile([128, NE * H], F32, tag=f"w32{nm}")
                nc.scalar.dma_start(
                    t32[:].rearrange("p (j h) -> p j h", h=H),
                    dram.ap().rearrange("(j p) h -> p j h", p=128),
                )
                w32[nm] = t32

            # ---- constants ----
            eye32 = cpool.tile([128, 128], F32, tag="eye32")
            make_identity(nc, eye32[:])
            eye_r = cpool.tile([128, 128], F32R, tag="eye_r")
            nc.vector.tensor_copy(eye_r[:], eye32[:])
            tri = cpool.tile([128, 128], BF16, tag="tri")
            make_upper_triangular(nc, tri[:], val=1.0, diag=True)

            # packed [Wq|Wk] lhsT tiles (bf16) and Wv (bf16)
            wqk = wpool.tile([128, NE * 128], BF16, tag="wqk")
            wvb = wpool.tile([128, NE * H], BF16, tag="wvb")
            for j in range(NE):
                nc.vector.tensor_copy(
                    wqk[:, 128 * j : 128 * j + 64],
                    w32["q"][:, H * j : H * j + H],
                )
                nc.vector.tensor_copy(
                    wqk[:, 128 * j + 64 : 128 * j + 128],
                    w32["k"][:, H * j : H * j + H],
                )
            nc.vector.tensor_copy(wvb[:], w32["v"][:])

            # ---- persistent tiles ----
            qTt = qkvpool.tile([64, T], BF16, tag="qTt")
            kTt = qkvpool.tile([64, T], BF16, tag="kTt")
            v1 = qkvpool.tile([128, NT * 65], BF16, tag="v1")
            # ones columns of v1 (col 64 of each 65-block)
            nc.vector.memset(
                v1[:].rearrange("p (i c) -> p i c", c=65)[:, :, 64:65], 1.0
            )
            xtg = [
                xtpool.tile([128, NE * 512], BF16, tag="xtg", name=f"xtg{g}")
                for g in range(NG)
            ]
            wT = [
                wtpool.tile([128, T], BF16, tag="wT", name=f"wT{j}")
                for j in range(NT)
            ]
            stage = [
                finpool.tile([128, 256], F32, tag="stage", bufs=2, name=f"st{g}")
                for g in range(NG)
            ]

            # ---------- emission unit generators ----------
            def t_units(g):
                """Transpose the 4 x-chunks of group g into xtg[g] (bf16)."""
                for c in range(4):
                    i = 4 * g + c
                    for jq in range(2):
                        def unit(i=i, c=c, jq=jq):
                            pt = pT.tile([128, 512], F32R, tag="pt")
                            for m in range(4):
                                j = 4 * jq + m
                                nc.tensor.transpose(
                                    pt[:, 128 * m : 128 * m + 128],
                                    xn[i][:, 128 * j : 128 * j + 128],
                                    eye_r[:],
                                )
                            # dest: 4 j-blocks at cols j*512 + c*128
                            dst = (
                                xtg[i // 4][:]
                                .rearrange("p (j t) -> p j t", t=512)[
                                    :, 4 * jq : 4 * jq + 4,
                                    128 * c : 128 * c + 128,
                                ]
                            )
                            nc.vector.tensor_copy(
                                dst, pt[:].rearrange("p (j t) -> p j t", t=128)
                            )
                        yield unit

            def qkvn_units(g):
                """qk-projection and v-projection for group g."""
                def qk_unit(jpair):
                    def unit():
                        if jpair == 0:
                            qkvn_units.pqk = pQK.tile([128, 512], F32, tag="pqk")
                        pqk = qkvn_units.pqk
                        for j in (2 * jpair, 2 * jpair + 1):
                            nc.tensor.matmul(
                                pqk[:],
                                wqk[:, 128 * j : 128 * j + 128],
                                xtg[g][:, 512 * j : 512 * j + 512],
                                start=(j == 0),
                                stop=(j == NE - 1),
                            )
                        if jpair == 3:
                            nc.vector.tensor_copy(
                                qTt[:, 512 * g : 512 * g + 512], pqk[0:64, :]
                            )
                            nc.scalar.copy(
                                kTt[:, 512 * g : 512 * g + 512], pqk[64:128, :]
                            )
                    return unit
                for jp in range(4):
                    yield qk_unit(jp)

                def v_unit(c):
                    def unit():
                        if c == 0:
                            qkvn_units.pv = pV.tile([128, 256], F32, tag="pv")
                        pv = qkvn_units.pv
                        for j in range(NE):
                            nc.tensor.matmul(
                                pv[:, 64 * c : 64 * c + 64],
                                xtg[g][:, 512 * j + 128 * c : 512 * j + 128 * c + 128],
                                wvb[:, 64 * j : 64 * j + 64],
                                start=(j == 0),
                                stop=(j == NE - 1),
                            )
                        if c == 3:
                            i0 = 4 * g
                            dst = (
                                v1[:]
                                .rearrange("p (i c) -> p i c", c=65)[
                                    :, i0 : i0 + 4, 0:64
                                ]
                            )
                            nc.scalar.copy(
                                dst, pv[:].rearrange("p (i c) -> p i c", c=64)
                            )
                    return unit
                for c in range(4):
                    yield v_unit(c)

            def s_units(g):
                """scoresT + exp for all s-chunks j <= 4g+3 over t-block g."""
                for j in range(4 * g + 4):
                    def unit(j=j, g=g):
                        off = max(0, 128 * j - 512 * g)  # diagonal trim
                        ps = pS.tile([128, 512], F32, tag="ps")
                        o = off
                        while o < 512:
                            n = min(512 - o, 512)
                            nc.tensor.matmul(
                                ps[:, o : o + n],
                                kTt[:, 128 * j : 128 * j + 128],
                                qTt[:, 512 * g + o : 512 * g + o + n],
                                start=True,
                                stop=True,
                            )
                            o += n
                        nc.scalar.activation(
                            wT[j][:, 512 * g + off : 512 * g + 512],
                            ps[:, off:512],
                            mybir.ActivationFunctionType.Exp,
                            scale=SCALE,
                        )
                        if 4 * g <= j:  # diagonal block: mask after exp
                            nc.vector.tensor_mul(
                                wT[j][:, 128 * j : 128 * j + 128],
                                wT[j][:, 128 * j : 128 * j + 128],
                                tri[:],
                            )
                    yield unit

            def av_units(g):
                """AV accumulation for the 4 t-chunks of group g (j-major:
                consecutive matmuls hit different psum col-slices so the
                accumulation chains pipeline) + normalization."""
                def alloc():
                    av_units.pav = pAV.tile([128, 260], F32, tag="pav")
                yield alloc
                for c in range(4):
                    i = 4 * g + c
                    js = list(range(i + 1))
                    batches = [js[k : k + 4] for k in range(0, len(js), 4)]
                    for bi, batch in enumerate(batches):
                        def unit(i=i, c=c, g=g, batch=batch,
                                 last=(bi == len(batches) - 1)):
                            pav = av_units.pav
                            for j in batch:
                                nc.tensor.matmul(
                                    pav[:, 65 * c : 65 * c + 65],
                                    wT[j][:, 128 * i : 128 * i + 128],
                                    v1[:, 65 * j : 65 * j + 65],
                                    start=(j == 0),
                                    stop=(j == i),
                                )
                            if last:
                                rcp = finpool.tile([128, 1], F32, tag="rcp",
                                                   bufs=4)
                                nc.vector.reciprocal(
                                    rcp[:], pav[:, 65 * c + 64 : 65 * c + 65]
                                )
                                nc.vector.tensor_scalar_mul(
                                    stage[g][:, 64 * c : 64 * c + 64],
                                    pav[:, 65 * c : 65 * c + 64],
                                    rcp[:],
                                )
                        yield unit
                def dma_out(g=g):
                    nc.sync.dma_start(
                        out_ap[512 * g : 512 * g + 512, :]
                        .rearrange("(c p) h -> p c h", p=128),
                        stage[g][:].rearrange("p (c h) -> p c h", h=64),
                    )
                yield dma_out

            def drain(*streams):
                streams = [s for s in streams if s is not None]
                while streams:
                    nxt = []
                    for s in streams:
                        u = next(s, None)
                        if u is not None:
                            u()
                            nxt.append(s)
                    streams = nxt

            # ---------- prologue: group 0 transpose + projections ----------
            drain(t_units(0))
            drain(qkvn_units(0))

            # ---------- steady loop ----------
            for g in range(NG):
                a = s_units(g)
                b = av_units(g - 1) if g >= 1 else None
                c = t_units(g + 1) if g + 1 < NG else None
                d = qkvn_units(g + 1) if g + 1 < NG else None
                # round-robin a/b with double-weight c; d after c drains
                streams = [s for s in (a, b) if s is not None]
                cd = c
                c_done = False
                while streams or cd is not None:
                    nxt = []
                    for s in streams:
                        u = next(s, None)
                        if u is not None:
                            u()
                            nxt.append(s)
                            if cd is None and s is a:
                                u = next(s, None)
                                if u is not None:
                                    u()
                    if cd is not None:
                        for _ in range(2):
                            u = next(cd, None)
                            if u is None:
                                if not c_done:
                                    c_done = True
                                    cd = d
                                else:
                                    cd = None
                                break
                            u()
                    streams = nxt

            # ---------- epilogue: AV of the last group ----------
            drain(av_units(NG - 1))

    if split_waits:
        _split_excess_waits(nc)
    return nc


_NC_CACHE = None


def _get_nc() -> bass.Bass:
    global _NC_CACHE
    if _NC_CACHE is None:
        _NC_CACHE = build_nc()
    return _NC_CACHE


def kernel(x, Wq, Wk, Wv, **run_kwargs):
    nc = _get_nc()
    x = np.ascontiguousarray(x, dtype=np.float32)
    in_maps = [
        {
            "x": np.ascontiguousarray(x[b]),
            "Wq": np.ascontiguousarray(Wq, dtype=np.float32),
            "Wk": np.ascontiguousarray(Wk, dtype=np.float32),
            "Wv": np.ascontiguousarray(Wv, dtype=np.float32),
        }
        for b in range(B)
    ]
    res = run_bass_kernel_spmd(nc, in_maps, core_ids=list(range(B)), **run_kwargs)
    out = np.stack([res.results[b]["out"] for b in range(B)], axis=0)
    kernel.last_results = res
    return out


# revision 26
# speedup vs baseline: 1.0213x; 1.0213x over previous
"""v4: single-head causal attention (B=8, T=2048, E=1024, H=64) on 8 trn2
cores, data-parallel over batch.

Pipeline per core (natural-v formulation, no final transpose):
  x f32 --HWDGE chunk DMA--> xn --PE f32r transpose--> psum --cast copy-->
  xT bf16 (group tiles, [j, t] layout)
  qkT[128, T] = [Wq|Wk]^T @ xT  (bf16, packed q rows 0:64 / k rows 64:128)
  v[t, 64]    = xT_chunk^T @ Wv (natural layout, ones col appended -> v1)
  scoresT[s-chunk, t] = kT_j^T @ qT  (bf16, diagonal-trimmed)
  wT = exp(scores/8) on ACT (psum -> sbuf bf16), diag masked by tri
  out_psum[t, 65] += wT_j^T @ v1_j   (col 64 = softmax denominator)
  out = psum[:, 0:64] * 1/psum[:, 64] on DVE -> staged -> DMA out

Emission is software-pipelined: loop g interleaves scores/exp(g),
AV(g-1), transposes(g+1), and projections(g+1) to keep PE dense (DVFS).
"""

import numpy as np

import concourse.bass as bass
import concourse.mybir as mybir
from concourse.tile import TileContext, add_dep_helper
from concourse.masks import make_identity, make_upper_triangular
from concourse.bass_utils import run_bass_kernel_spmd

B, T, E, H = 8, 2048, 1024, 64
NT = T // 128   # 16 t-chunks
NE = E // 128   # 8 e-chunks
NG = 4          # t-groups of 4 chunks / 512 cols
F32 = mybir.dt.float32
F32R = mybir.dt.float32r
BF16 = mybir.dt.bfloat16
SCALE = float(H) ** -0.5


def _split_excess_waits(nc: bass.Bass, cap: int = 1) -> int:
    n_split = 0
    for f in nc.m.functions:
        for bb in f.blocks:
            insts = list(bb.instructions)
            out = []
            dirty = False
            for inst in insts:
                si = inst.sync_info
                waits = list(si.on_wait) if si and si.on_wait else []
                if len(waits) > cap:
                    si.on_wait = waits[:cap]
                    for w in waits[cap:]:
                        nop = mybir.InstNoOp(
                            name=f"I-waitsplit-{n_split}", ins=[], outs=[]
                        )
                        nop.engine = inst.engine
                        nop.sync_info = mybir.SyncInfo(on_wait=[w], on_update=[])
                        out.append(nop)
                        n_split += 1
                    dirty = True
                out.append(inst)
            if dirty:
                bb.instructions = out
    return n_split


def build_nc(split_waits: bool = True) -> bass.Bass:
    nc = bass.Bass()
    x = nc.dram_tensor("x", [T, E], F32R, kind="ExternalInput")
    wq = nc.dram_tensor("Wq", [E, H], F32, kind="ExternalInput")
    wk = nc.dram_tensor("Wk", [E, H], F32, kind="ExternalInput")
    wv = nc.dram_tensor("Wv", [E, H], F32, kind="ExternalInput")
    out = nc.dram_tensor("out", [T, H], F32, kind="ExternalOutput")
    x_ap, out_ap = x.ap(), out.ap()

    with TileContext(nc) as tc:
        with (
            tc.tile_pool(name="const", bufs=1) as cpool,
            tc.tile_pool(name="wts", bufs=1) as wpool,
            tc.tile_pool(name="xn", bufs=6) as xnpool,
            tc.tile_pool(name="xtg", bufs=2) as xtpool,
            tc.tile_pool(name="qkv", bufs=1) as qkvpool,
            tc.tile_pool(name="wT", bufs=16) as wtpool,
            tc.tile_pool(name="fin", bufs=8) as finpool,
            tc.tile_pool(name="pT", bufs=2, space="PSUM") as pT,
            tc.tile_pool(name="pQK", bufs=1, space="PSUM") as pQK,
            tc.tile_pool(name="pV", bufs=1, space="PSUM") as pV,
            tc.tile_pool(name="pS", bufs=3, space="PSUM") as pS,
            tc.tile_pool(name="pAV", bufs=1, space="PSUM") as pAV,
        ):
            # ---- input DMAs first: x chunks alternate both HWDGE rings ----
            xn = []
            xdmas = []
            for i in range(NT):
                t_ = xnpool.tile([128, E], F32R, tag="xn", name=f"xn{i}")
                if i == 0:
                    nc.sync.dma_start(t_[:, 0:512], x_ap[0:128, 0:512])
                    d = nc.sync.dma_start(t_[:, 512:E], x_ap[0:128, 512:E])
                else:
                    d = nc.sync.dma_start(
                        t_[:], x_ap[128 * i : 128 * i + 128, :]
                    )
                if i >= 2:
                    add_dep_helper(
                        d.ins, xdmas[i - 2].ins, sync=True,
                        reason="pace x chunk DMAs for in-order completion",
                    )
                xdmas.append(d)
                xn.append(t_)
            # weights on the scalar (ACT) HWDGE ring
            w32 = {}
            for nm, dram in (("q", wq), ("k", wk), ("v", wv)):
                t32 = wpool.tile([128, NE * H], F32, tag=f"w32{nm}")
                nc.scalar.dma_start(
                    t32[:].rearrange("p (j h) -> p j h", h=H),
                    dram.ap().rearrange("(j p) h -> p j h", p=128),
                )
                w32[nm] = t32

            # ---- constants ----
            eye32 = cpool.tile([128, 128], F32, tag="eye32")
            make_identity(nc, eye32[:])
            eye_r = cpool.tile([128, 128], F32R, tag="eye_r")
            nc.vector.tensor_copy(eye_r[:], eye32[:])
            tri = cpool.tile([128, 128], BF16, tag="tri")
            make_upper_triangular(nc, tri[:], val=1.0, diag=True)

            # packed [Wq|Wk] lhsT tiles (bf16) and Wv (bf16)
            wqk = wpool.tile([128, NE * 128], BF16, tag="wqk")
            wvb = wpool.tile([128, NE * H], BF16, tag="wvb")
            for j in range(NE):
                nc.vector.tensor_copy(
                    wqk[:, 128 * j : 128 * j + 64],
                    w32["q"][:, H * j : H * j + H],
                )
                nc.vector.tensor_copy(
                    wqk[:, 128 * j + 64 : 128 * j + 128],
                    w32["k"][:, H * j : H * j + H],
                )
            nc.vector.tensor_copy(wvb[:], w32["v"][:])

            # ---- persistent tiles ----
            qTt = qkvpool.tile([64, T], BF16, tag="qTt")
            kTt = qkvpool.tile([64, T], BF16, tag="kTt")
            v1 = qkvpool.tile([128, NT * 65], BF16, tag="v1")
            # ones columns of v1 (col 64 of each 65-block)
            nc.vector.memset(
                v1[:].rearrange("p (i c) -> p i c", c=65)[:, :, 64:65], 1.0
            )
            xtg = [
                xtpool.tile([128, NE * 512], BF16, tag="xtg", name=f"xtg{g}")
                for g in range(NG)
            ]
            wT = [
                wtpool.tile([128, T], BF16, tag="wT", name=f"wT{j}")
                for j in range(NT)
            ]
            stage = [
                finpool.tile([128, 256], F32, tag="stage", bufs=2, name=f"st{g}")
                for g in range(NG)
            ]

            # ---------- emission unit generators ----------
            def t_units(g):
                """Transpose the 4 x-chunks of group g into xtg[g] (bf16)."""
                for c in range(4):
                    i = 4 * g + c
                    for jq in range(2):
                        def unit(i=i, c=c, jq=jq):
                            pt = pT.tile([128, 512], F32R, tag="pt")
                            for m in range(4):
                                j = 4 * jq + m
                                nc.tensor.transpose(
                                    pt[:, 128 * m : 128 * m + 128],
                                    xn[i][:, 128 * j : 128 * j + 128],
                                    eye_r[:],
                                )
                            # dest: 4 j-blocks at cols j*512 + c*128
                            dst = (
                                xtg[i // 4][:]
                                .rearrange("p (j t) -> p j t", t=512)[
                                    :, 4 * jq : 4 * jq + 4,
                                    128 * c : 128 * c + 128,
                                ]
                            )
                            nc.vector.tensor_copy(
                                dst, pt[:].rearrange("p (j t) -> p j t", t=128)
                            )
                        yield unit

            def qkvn_units(g):
                """qk-projection and v-projection for group g."""
                def qk_unit(jpair):
                    def unit():
                        if jpair == 0:
                            qkvn_units.pqk = pQK.tile([128, 512], F32, tag="pqk")
                        pqk = qkvn_units.pqk
                        for j in (2 * jpair, 2 * jpair + 1):
                            nc.tensor.matmul(
                                pqk[:],
                                wqk[:, 128 * j : 128 * j + 128],
                                xtg[g][:, 512 * j : 512 * j + 512],
                                start=(j == 0),
                                stop=(j == NE - 1),
                            )
                        if jpair == 3:
                            nc.vector.tensor_copy(
                                qTt[:, 512 * g : 512 * g + 512], pqk[0:64, :]
                            )
                            nc.scalar.copy(
                                kTt[:, 512 * g : 512 * g + 512], pqk[64:128, :]
                            )
                    return unit
                for jp in range(4):
                    yield qk_unit(jp)

                def v_unit(c):
                    def unit():
                        if c == 0:
                            qkvn_units.pv = pV.tile([128, 256], F32, tag="pv")
                        pv = qkvn_units.pv
                        for j in range(NE):
                            nc.tensor.matmul(
                                pv[:, 64 * c : 64 * c + 64],
                                xtg[g][:, 512 * j + 128 * c : 512 * j + 128 * c + 128],
                                wvb[:, 64 * j : 64 * j + 64],
                                start=(j == 0),
                                stop=(j == NE - 1),
                            )
                        if c == 3:
                            i0 = 4 * g
                            dst = (
                                v1[:]
                                .rearrange("p (i c) -> p i c", c=65)[
                                    :, i0 : i0 + 4, 0:64
                                ]
                            )
                            nc.scalar.copy(
                                dst, pv[:].rearrange("p (i c) -> p i c", c=64)
                            )
                    return unit
                for c in range(4):
                    yield v_unit(c)

            def s_units(g):
                """scoresT + exp for all s-chunks j <= 4g+3 over t-block g."""
                for j in range(4 * g + 4):
                    def unit(j=j, g=g):
                        off = max(0, 128 * j - 512 * g)  # diagonal trim
                        ps = pS.tile([128, 512], F32, tag="ps")
                        o = off
                        while o < 512:
                            n = min(512 - o, 512)
                            nc.tensor.matmul(
                                ps[:, o : o + n],
                                kTt[:, 128 * j : 128 * j + 128],
                                qTt[:, 512 * g + o : 512 * g + o + n],
                                start=True,
                                stop=True,
                            )
                            o += n
                        nc.scalar.activation(
                            wT[j][:, 512 * g + off : 512 * g + 512],
                            ps[:, off:512],
                            mybir.ActivationFunctionType.Exp,
                            scale=SCALE,
                        )
                        if 4 * g <= j:  # diagonal block: mask after exp
                            nc.vector.tensor_mul(
                                wT[j][:, 128 * j : 128 * j + 128],
                                wT[j][:, 128 * j : 128 * j + 128],
                                tri[:],
                            )
                    yield unit

            def av_units(g):
                """AV accumulation for the 4 t-chunks of group g (j-major:
                consecutive matmuls hit different psum col-slices so the
                accumulation chains pipeline) + normalization."""
                def alloc():
                    av_units.pav = pAV.tile([128, 260], F32, tag="pav")
                yield alloc
                for c in range(4):
                    i = 4 * g + c
                    js = list(range(i + 1))
                    batches = [js[k : k + 4] for k in range(0, len(js), 4)]
                    for bi, batch in enumerate(batches):
                        def unit(i=i, c=c, g=g, batch=batch,
                                 last=(bi == len(batches) - 1)):
                            pav = av_units.pav
                            for j in batch:
                                nc.tensor.matmul(
                                    pav[:, 65 * c : 65 * c + 65],
                                    wT[j][:, 128 * i : 128 * i + 128],
                                    v1[:, 65 * j : 65 * j + 65],
                                    start=(j == 0),
                                    stop=(j == i),
                                )
                            if last:
                                rcp = finpool.tile([128, 1], F32, tag="rcp",
                                                   bufs=4)
                                nc.vector.reciprocal(
                                    rcp[:], pav[:, 65 * c + 64 : 65 * c + 65]
                                )
                                nc.vector.tensor_scalar_mul(
                                    stage[g][:, 64 * c : 64 * c + 64],
                                    pav[:, 65 * c : 65 * c + 64],
                                    rcp[:],
                                )
                        yield unit
                def dma_out(g=g):
                    nc.sync.dma_start(
                        out_ap[512 * g : 512 * g + 512, :]
                        .rearrange("(c p) h -> p c h", p=128),
                        stage[g][:].rearrange("p (c h) -> p c h", h=64),
                    )
                yield dma_out

            def drain(*streams):
                streams = [s for s in streams if s is not None]
                while streams:
                    nxt = []
                    for s in streams:
                        u = next(s, None)
                        if u is not None:
                            u()
                            nxt.append(s)
                    streams = nxt

            # ---------- prologue: group 0 transpose + projections ----------
            drain(t_units(0))
            drain(qkvn_units(0))

            # ---------- steady loop ----------
            for g in range(NG):
                a = s_units(g)
                b = av_units(g - 1) if g >= 1 else None
                c = t_units(g + 1) if g + 1 < NG else None
                d = qkvn_units(g + 1) if g + 1 < NG else None
                # round-robin a/b with double-weight c; d after c drains
                streams = [s for s in (a, b) if s is not None]
                cd = c
                c_done = False
                while streams or cd is not None:
                    nxt = []
                    for s in streams:
                        u = next(s, None)
                        if u is not None:
                            u()
                            nxt.append(s)
                            if cd is None and s is a:
                                u = next(s, None)
                                if u is not None:
                                    u()
                    if cd is not None:
                        for _ in range(2):
                            u = next(cd, None)
                            if u is None:
                                if not c_done:
                                    c_done = True
                                    cd = d
                                else:
                                    cd = None
                                break
                            u()
                    streams = nxt

            # ---------- epilogue: AV of the last group ----------
            drain(av_units(NG - 1))

    if split_waits:
        _split_excess_waits(nc)
    return nc


_NC_CACHE = None


def _get_nc() -> bass.Bass:
    global _NC_CACHE
    if _NC_CACHE is None:
        _NC_CACHE = build_nc()
    return _NC_CACHE


def kernel(x, Wq, Wk, Wv, **run_kwargs):
    nc = _get_nc()
    x = np.ascontiguousarray(x, dtype=np.float32)
    in_maps = [
        {
            "x": np.ascontiguousarray(x[b]),
            "Wq": np.ascontiguousarray(Wq, dtype=np.float32),
            "Wk": np.ascontiguousarray(Wk, dtype=np.float32),
            "Wv": np.ascontiguousarray(Wv, dtype=np.float32),
        }
        for b in range(B)
    ]
    res = run_bass_kernel_spmd(nc, in_maps, core_ids=list(range(B)), **run_kwargs)
    out = np.stack([res.results[b]["out"] for b in range(B)], axis=0)
    kernel.last_results = res
    return out


# revision 27
# speedup vs baseline: 1.0368x; 1.0152x over previous
"""v4: single-head causal attention (B=8, T=2048, E=1024, H=64) on 8 trn2
cores, data-parallel over batch.

Pipeline per core (natural-v formulation, no final transpose):
  x f32 --HWDGE chunk DMA--> xn --PE f32r transpose--> psum --cast copy-->
  xT bf16 (group tiles, [j, t] layout)
  qkT[128, T] = [Wq|Wk]^T @ xT  (bf16, packed q rows 0:64 / k rows 64:128)
  v[t, 64]    = xT_chunk^T @ Wv (natural layout, ones col appended -> v1)
  scoresT[s-chunk, t] = kT_j^T @ qT  (bf16, diagonal-trimmed)
  wT = exp(scores/8) on ACT (psum -> sbuf bf16), diag masked by tri
  out_psum[t, 65] += wT_j^T @ v1_j   (col 64 = softmax denominator)
  out = psum[:, 0:64] * 1/psum[:, 64] on DVE -> staged -> DMA out

Emission is software-pipelined: loop g interleaves scores/exp(g),
AV(g-1), transposes(g+1), and projections(g+1) to keep PE dense (DVFS).
"""

import numpy as np

import concourse.bass as bass
import concourse.mybir as mybir
from concourse.tile import TileContext, add_dep_helper
from concourse.masks import make_identity, make_upper_triangular
from concourse.bass_utils import run_bass_kernel_spmd

B, T, E, H = 8, 2048, 1024, 64
NT = T // 128   # 16 t-chunks
NE = E // 128   # 8 e-chunks
NG = 4          # t-groups of 4 chunks / 512 cols
F32 = mybir.dt.float32
F32R = mybir.dt.float32r
BF16 = mybir.dt.bfloat16
SCALE = float(H) ** -0.5


def _split_excess_waits(nc: bass.Bass, cap: int = 1) -> int:
    n_split = 0
    for f in nc.m.functions:
        for bb in f.blocks:
            insts = list(bb.instructions)
            out = []
            dirty = False
            for inst in insts:
                si = inst.sync_info
                waits = list(si.on_wait) if si and si.on_wait else []
                if len(waits) > cap:
                    si.on_wait = waits[:cap]
                    for w in waits[cap:]:
                        nop = mybir.InstNoOp(
                            name=f"I-waitsplit-{n_split}", ins=[], outs=[]
                        )
                        nop.engine = inst.engine
                        nop.sync_info = mybir.SyncInfo(on_wait=[w], on_update=[])
                        out.append(nop)
                        n_split += 1
                    dirty = True
                out.append(inst)
            if dirty:
                bb.instructions = out
    return n_split


def build_nc(split_waits: bool = True) -> bass.Bass:
    nc = bass.Bass()
    x = nc.dram_tensor("x", [T, E], F32R, kind="ExternalInput")
    wq = nc.dram_tensor("Wq", [E, H], F32, kind="ExternalInput")
    wk = nc.dram_tensor("Wk", [E, H], F32, kind="ExternalInput")
    wv = nc.dram_tensor("Wv", [E, H], F32, kind="ExternalInput")
    out = nc.dram_tensor("out", [T, H], F32, kind="ExternalOutput")
    x_ap, out_ap = x.ap(), out.ap()

    with TileContext(nc) as tc:
        with (
            tc.tile_pool(name="const", bufs=1) as cpool,
            tc.tile_pool(name="wts", bufs=1) as wpool,
            tc.tile_pool(name="xn", bufs=6) as xnpool,
            tc.tile_pool(name="xtg", bufs=2) as xtpool,
            tc.tile_pool(name="qkv", bufs=1) as qkvpool,
            tc.tile_pool(name="wT", bufs=16) as wtpool,
            tc.tile_pool(name="fin", bufs=8) as finpool,
            tc.tile_pool(name="pT", bufs=2, space="PSUM") as pT,
            tc.tile_pool(name="pQK", bufs=1, space="PSUM") as pQK,
            tc.tile_pool(name="pV", bufs=1, space="PSUM") as pV,
            tc.tile_pool(name="pS", bufs=3, space="PSUM") as pS,
            tc.tile_pool(name="pAV", bufs=1, space="PSUM") as pAV,
        ):
            # ---- input DMAs first: x chunks alternate both HWDGE rings ----
            xn = []
            xdmas = []
            for i in range(NT):
                t_ = xnpool.tile([128, E], F32R, tag="xn", name=f"xn{i}")
                if i == 0:
                    nc.sync.dma_start(t_[:, 0:512], x_ap[0:128, 0:512])
                    d = nc.sync.dma_start(t_[:, 512:E], x_ap[0:128, 512:E])
                else:
                    d = nc.sync.dma_start(
                        t_[:], x_ap[128 * i : 128 * i + 128, :]
                    )
                if i >= 3:
                    add_dep_helper(
                        d.ins, xdmas[i - 3].ins, sync=True,
                        reason="pace x chunk DMAs for in-order completion",
                    )
                xdmas.append(d)
                xn.append(t_)
            # weights on the scalar (ACT) HWDGE ring
            w32 = {}
            for nm, dram in (("q", wq), ("k", wk), ("v", wv)):
                t32 = wpool.tile([128, NE * H], F32, tag=f"w32{nm}")
                nc.scalar.dma_start(
                    t32[:].rearrange("p (j h) -> p j h", h=H),
                    dram.ap().rearrange("(j p) h -> p j h", p=128),
                )
                w32[nm] = t32

            # ---- constants ----
            eye32 = cpool.tile([128, 128], F32, tag="eye32")
            make_identity(nc, eye32[:])
            eye_r = cpool.tile([128, 128], F32R, tag="eye_r")
            nc.vector.tensor_copy(eye_r[:], eye32[:])
            tri = cpool.tile([128, 128], BF16, tag="tri")
            make_upper_triangular(nc, tri[:], val=1.0, diag=True)

            # packed [Wq|Wk] lhsT tiles (bf16) and Wv (bf16)
            wqk = wpool.tile([128, NE * 128], BF16, tag="wqk")
            wvb = wpool.tile([128, NE * H], BF16, tag="wvb")
            for j in range(NE):
                nc.vector.tensor_copy(
                    wqk[:, 128 * j : 128 * j + 64],
                    w32["q"][:, H * j : H * j + H],
                )
                nc.vector.tensor_copy(
                    wqk[:, 128 * j + 64 : 128 * j + 128],
                    w32["k"][:, H * j : H * j + H],
                )
            nc.vector.tensor_copy(wvb[:], w32["v"][:])

            # ---- persistent tiles ----
            qTt = qkvpool.tile([64, T], BF16, tag="qTt")
            kTt = qkvpool.tile([64, T], BF16, tag="kTt")
            v1 = qkvpool.tile([128, NT * 65], BF16, tag="v1")
            # ones columns of v1 (col 64 of each 65-block)
            nc.vector.memset(
                v1[:].rearrange("p (i c) -> p i c", c=65)[:, :, 64:65], 1.0
            )
            xtg = [
                xtpool.tile([128, NE * 512], BF16, tag="xtg", name=f"xtg{g}")
                for g in range(NG)
            ]
            wT = [
                wtpool.tile([128, T], BF16, tag="wT", name=f"wT{j}")
                for j in range(NT)
            ]
            stage = [
                finpool.tile([128, 256], F32, tag="stage", bufs=2, name=f"st{g}")
                for g in range(NG)
            ]

            # ---------- emission unit generators ----------
            def t_units(g):
                """Transpose the 4 x-chunks of group g into xtg[g] (bf16)."""
                for c in range(4):
                    i = 4 * g + c
                    for jq in range(2):
                        def unit(i=i, c=c, jq=jq):
                            pt = pT.tile([128, 512], F32R, tag="pt")
                            for m in range(4):
                                j = 4 * jq + m
                                nc.tensor.transpose(
                                    pt[:, 128 * m : 128 * m + 128],
                                    xn[i][:, 128 * j : 128 * j + 128],
                                    eye_r[:],
                                )
                            # dest: 4 j-blocks at cols j*512 + c*128
                            dst = (
                                xtg[i // 4][:]
                                .rearrange("p (j t) -> p j t", t=512)[
                                    :, 4 * jq : 4 * jq + 4,
                                    128 * c : 128 * c + 128,
                                ]
                            )
                            nc.vector.tensor_copy(
                                dst, pt[:].rearrange("p (j t) -> p j t", t=128)
                            )
                        yield unit

            def qkvn_units(g):
                """qk-projection and v-projection for group g."""
                def qk_unit(jpair):
                    def unit():
                        if jpair == 0:
                            qkvn_units.pqk = pQK.tile([128, 512], F32, tag="pqk")
                        pqk = qkvn_units.pqk
                        for j in (2 * jpair, 2 * jpair + 1):
                            nc.tensor.matmul(
                                pqk[:],
                                wqk[:, 128 * j : 128 * j + 128],
                                xtg[g][:, 512 * j : 512 * j + 512],
                                start=(j == 0),
                                stop=(j == NE - 1),
                            )
                        if jpair == 3:
                            nc.vector.tensor_copy(
                                qTt[:, 512 * g : 512 * g + 512], pqk[0:64, :]
                            )
                            nc.scalar.copy(
                                kTt[:, 512 * g : 512 * g + 512], pqk[64:128, :]
                            )
                    return unit
                for jp in range(4):
                    yield qk_unit(jp)

                def v_unit(c):
                    def unit():
                        if c == 0:
                            qkvn_units.pv = pV.tile([128, 256], F32, tag="pv")
                        pv = qkvn_units.pv
                        for j in range(NE):
                            nc.tensor.matmul(
                                pv[:, 64 * c : 64 * c + 64],
                                xtg[g][:, 512 * j + 128 * c : 512 * j + 128 * c + 128],
                                wvb[:, 64 * j : 64 * j + 64],
                                start=(j == 0),
                                stop=(j == NE - 1),
                            )
                        if c == 3:
                            i0 = 4 * g
                            dst = (
                                v1[:]
                                .rearrange("p (i c) -> p i c", c=65)[
                                    :, i0 : i0 + 4, 0:64
                                ]
                            )
                            nc.scalar.copy(
                                dst, pv[:].rearrange("p (i c) -> p i c", c=64)
                            )
                    return unit
                for c in range(4):
                    yield v_unit(c)

            def s_units(g):
                """scoresT + exp for all s-chunks j <= 4g+3 over t-block g."""
                for j in range(4 * g + 4):
                    def unit(j=j, g=g):
                        off = max(0, 128 * j - 512 * g)  # diagonal trim
                        ps = pS.tile([128, 512], F32, tag="ps")
                        o = off
                        while o < 512:
                            n = min(512 - o, 512)
                            nc.tensor.matmul(
                                ps[:, o : o + n],
                                kTt[:, 128 * j : 128 * j + 128],
                                qTt[:, 512 * g + o : 512 * g + o + n],
                                start=True,
                                stop=True,
                            )
                            o += n
                        nc.scalar.activation(
                            wT[j][:, 512 * g + off : 512 * g + 512],
                            ps[:, off:512],
                            mybir.ActivationFunctionType.Exp,
                            scale=SCALE,
                        )
                        if 4 * g <= j:  # diagonal block: mask after exp
                            nc.vector.tensor_mul(
                                wT[j][:, 128 * j : 128 * j + 128],
                                wT[j][:, 128 * j : 128 * j + 128],
                                tri[:],
                            )
                    yield unit

            def av_units(g):
                """AV accumulation for the 4 t-chunks of group g (j-major:
                consecutive matmuls hit different psum col-slices so the
                accumulation chains pipeline) + normalization."""
                def alloc():
                    av_units.pav = pAV.tile([128, 260], F32, tag="pav")
                yield alloc
                for c in range(4):
                    i = 4 * g + c
                    js = list(range(i + 1))
                    batches = [js[k : k + 4] for k in range(0, len(js), 4)]
                    for bi, batch in enumerate(batches):
                        def unit(i=i, c=c, g=g, batch=batch,
                                 last=(bi == len(batches) - 1)):
                            pav = av_units.pav
                            for j in batch:
                                nc.tensor.matmul(
                                    pav[:, 65 * c : 65 * c + 65],
                                    wT[j][:, 128 * i : 128 * i + 128],
                                    v1[:, 65 * j : 65 * j + 65],
                                    start=(j == 0),
                                    stop=(j == i),
                                )
                            if last:
                                rcp = finpool.tile([128, 1], F32, tag="rcp",
                                                   bufs=4)
                                nc.vector.reciprocal(
                                    rcp[:], pav[:, 65 * c + 64 : 65 * c + 65]
                                )
                                nc.vector.tensor_scalar_mul(
                                    stage[g][:, 64 * c : 64 * c + 64],
                                    pav[:, 65 * c : 65 * c + 64],
                                    rcp[:],
                                )
                        yield unit
                def dma_out(g=g):
                    nc.sync.dma_start(
                        out_ap[512 * g : 512 * g + 512, :]
                        .rearrange("(c p) h -> p c h", p=128),
                        stage[g][:].rearrange("p (c h) -> p c h", h=64),
                    )
                yield dma_out

            def drain(*streams):
                streams = [s for s in streams if s is not None]
                while streams:
                    nxt = []
                    for s in streams:
                        u = next(s, None)
                        if u is not None:
                            u()
                            nxt.append(s)
                    streams = nxt

            # ---------- prologue: group 0 transpose + projections ----------
            drain(t_units(0))
            drain(qkvn_units(0))

            # ---------- steady loop ----------
            for g in range(NG):
                a = s_units(g)
                b = av_units(g - 1) if g >= 1 else None
                c = t_units(g + 1) if g + 1 < NG else None
                d = qkvn_units(g + 1) if g + 1 < NG else None
                # round-robin a/b with double-weight c; d after c drains
                streams = [s for s in (a, b) if s is not None]
                cd = c
                c_done = False
                while streams or cd is not None:
                    nxt = []
                    for s in streams:
                        u = next(s, None)
                        if u is not None:
                            u()
                            nxt.append(s)
                            if cd is None and s is a:
                                u = next(s, None)
                                if u is not None:
                                    u()
                    if cd is not None:
                        for _ in range(2):
                            u = next(cd, None)
                            if u is None:
                                if not c_done:
                                    c_done = True
                                    cd = d
                                else:
                                    cd = None
                                break
                            u()
                    streams = nxt

            # ---------- epilogue: AV of the last group ----------
            drain(av_units(NG - 1))

    if split_waits:
        _split_excess_waits(nc)
    return nc


_NC_CACHE = None


def _get_nc() -> bass.Bass:
    global _NC_CACHE
    if _NC_CACHE is None:
        _NC_CACHE = build_nc()
    return _NC_CACHE


def kernel(x, Wq, Wk, Wv, **run_kwargs):
    nc = _get_nc()
    x = np.ascontiguousarray(x, dtype=np.float32)
    in_maps = [
        {
            "x": np.ascontiguousarray(x[b]),
            "Wq": np.ascontiguousarray(Wq, dtype=np.float32),
            "Wk": np.ascontiguousarray(Wk, dtype=np.float32),
            "Wv": np.ascontiguousarray(Wv, dtype=np.float32),
        }
        for b in range(B)
    ]
    res = run_bass_kernel_spmd(nc, in_maps, core_ids=list(range(B)), **run_kwargs)
    out = np.stack([res.results[b]["out"] for b in range(B)], axis=0)
    kernel.last_results = res
    return out


# revision 28
# speedup vs baseline: 1.0939x; 1.0551x over previous
"""v4: single-head causal attention (B=8, T=2048, E=1024, H=64) on 8 trn2
cores, data-parallel over batch.

Pipeline per core (natural-v formulation, no final transpose):
  x f32 --HWDGE chunk DMA--> xn --PE f32r transpose--> psum --cast copy-->
  xT bf16 (group tiles, [j, t] layout)
  qkT[128, T] = [Wq|Wk]^T @ xT  (bf16, packed q rows 0:64 / k rows 64:128)
  v[t, 64]    = xT_chunk^T @ Wv (natural layout, ones col appended -> v1)
  scoresT[s-chunk, t] = kT_j^T @ qT  (bf16, diagonal-trimmed)
  wT = exp(scores/8) on ACT (psum -> sbuf bf16), diag masked by tri
  out_psum[t, 65] += wT_j^T @ v1_j   (col 64 = softmax denominator)
  out = psum[:, 0:64] * 1/psum[:, 64] on DVE -> staged -> DMA out

Emission is software-pipelined: loop g interleaves scores/exp(g),
AV(g-1), transposes(g+1), and projections(g+1) to keep PE dense (DVFS).
"""

import numpy as np

import concourse.bass as bass
import concourse.mybir as mybir
from concourse.tile import TileContext, add_dep_helper
from concourse.masks import make_identity, make_upper_triangular
from concourse.bass_utils import run_bass_kernel_spmd

B, T, E, H = 8, 2048, 1024, 64
NT = T // 128   # 16 t-chunks
NE = E // 128   # 8 e-chunks
NG = 4          # t-groups of 4 chunks / 512 cols
F32 = mybir.dt.float32
F32R = mybir.dt.float32r
BF16 = mybir.dt.bfloat16
SCALE = float(H) ** -0.5


def _split_excess_waits(nc: bass.Bass, cap: int = 1) -> int:
    n_split = 0
    for f in nc.m.functions:
        for bb in f.blocks:
            insts = list(bb.instructions)
            out = []
            dirty = False
            for inst in insts:
                si = inst.sync_info
                waits = list(si.on_wait) if si and si.on_wait else []
                if len(waits) > cap:
                    si.on_wait = waits[:cap]
                    for w in waits[cap:]:
                        nop = mybir.InstNoOp(
                            name=f"I-waitsplit-{n_split}", ins=[], outs=[]
                        )
                        nop.engine = inst.engine
                        nop.sync_info = mybir.SyncInfo(on_wait=[w], on_update=[])
                        out.append(nop)
                        n_split += 1
                    dirty = True
                out.append(inst)
            if dirty:
                bb.instructions = out
    return n_split


def build_nc(split_waits: bool = True) -> bass.Bass:
    nc = bass.Bass()
    x = nc.dram_tensor("x", [T, E], F32R, kind="ExternalInput")
    wq = nc.dram_tensor("Wq", [E, H], F32, kind="ExternalInput")
    wk = nc.dram_tensor("Wk", [E, H], F32, kind="ExternalInput")
    wv = nc.dram_tensor("Wv", [E, H], F32, kind="ExternalInput")
    out = nc.dram_tensor("out", [T, H], F32, kind="ExternalOutput")
    x_ap, out_ap = x.ap(), out.ap()

    with TileContext(nc) as tc:
        with (
            tc.tile_pool(name="const", bufs=1) as cpool,
            tc.tile_pool(name="wts", bufs=1) as wpool,
            tc.tile_pool(name="xn", bufs=6) as xnpool,
            tc.tile_pool(name="xtg", bufs=2) as xtpool,
            tc.tile_pool(name="qkv", bufs=1) as qkvpool,
            tc.tile_pool(name="wT", bufs=16) as wtpool,
            tc.tile_pool(name="fin", bufs=8) as finpool,
            tc.tile_pool(name="pT", bufs=2, space="PSUM") as pT,
            tc.tile_pool(name="pQK", bufs=1, space="PSUM") as pQK,
            tc.tile_pool(name="pV", bufs=1, space="PSUM") as pV,
            tc.tile_pool(name="pS", bufs=3, space="PSUM") as pS,
            tc.tile_pool(name="pAV", bufs=1, space="PSUM") as pAV,
        ):
            # ---- input DMAs first: x chunks alternate both HWDGE rings ----
            xn = []
            xdmas = []
            for i in range(NT):
                t_ = xnpool.tile([128, E], F32R, tag="xn", name=f"xn{i}")
                r = x_ap[128 * i : 128 * i + 128, :]
                for h in range(2):
                    d = nc.sync.dma_start(
                        t_[:, 512 * h : 512 * h + 512],
                        r[:, 512 * h : 512 * h + 512],
                    )
                    k = 2 * i + h
                    if k >= 6:
                        add_dep_helper(
                            d.ins, xdmas[k - 6].ins, sync=True,
                            reason="pace x chunk DMAs for in-order completion",
                        )
                    xdmas.append(d)
                xn.append(t_)
            # weights on the scalar (ACT) HWDGE ring
            w32 = {}
            for nm, dram in (("q", wq), ("k", wk), ("v", wv)):
                t32 = wpool.tile([128, NE * H], F32, tag=f"w32{nm}")
                nc.scalar.dma_start(
                    t32[:].rearrange("p (j h) -> p j h", h=H),
                    dram.ap().rearrange("(j p) h -> p j h", p=128),
                )
                w32[nm] = t32

            # ---- constants ----
            eye32 = cpool.tile([128, 128], F32, tag="eye32")
            make_identity(nc, eye32[:])
            eye_r = cpool.tile([128, 128], F32R, tag="eye_r")
            nc.vector.tensor_copy(eye_r[:], eye32[:])
            tri = cpool.tile([128, 128], BF16, tag="tri")
            make_upper_triangular(nc, tri[:], val=1.0, diag=True)

            # packed [Wq|Wk] lhsT tiles (bf16) and Wv (bf16)
            wqk = wpool.tile([128, NE * 128], BF16, tag="wqk")
            wvb = wpool.tile([128, NE * H], BF16, tag="wvb")
            for j in range(NE):
                nc.vector.tensor_copy(
                    wqk[:, 128 * j : 128 * j + 64],
                    w32["q"][:, H * j : H * j + H],
                )
                nc.vector.tensor_copy(
                    wqk[:, 128 * j + 64 : 128 * j + 128],
                    w32["k"][:, H * j : H * j + H],
                )
            nc.vector.tensor_copy(wvb[:], w32["v"][:])

            # ---- persistent tiles ----
            qTt = qkvpool.tile([64, T], BF16, tag="qTt")
            kTt = qkvpool.tile([64, T], BF16, tag="kTt")
            v1 = qkvpool.tile([128, NT * 65], BF16, tag="v1")
            # ones columns of v1 (col 64 of each 65-block)
            nc.vector.memset(
                v1[:].rearrange("p (i c) -> p i c", c=65)[:, :, 64:65], 1.0
            )
            xtg = [
                xtpool.tile([128, NE * 512], BF16, tag="xtg", name=f"xtg{g}")
                for g in range(NG)
            ]
            wT = [
                wtpool.tile([128, T], BF16, tag="wT", name=f"wT{j}")
                for j in range(NT)
            ]
            stage = [
                finpool.tile([128, 256], F32, tag="stage", bufs=2, name=f"st{g}")
                for g in range(NG)
            ]

            # ---------- emission unit generators ----------
            def t_units(g):
                """Transpose the 4 x-chunks of group g into xtg[g] (bf16)."""
                for c in range(4):
                    i = 4 * g + c
                    for jq in range(2):
                        def unit(i=i, c=c, jq=jq):
                            pt = pT.tile([128, 512], F32R, tag="pt")
                            for m in range(4):
                                j = 4 * jq + m
                                nc.tensor.transpose(
                                    pt[:, 128 * m : 128 * m + 128],
                                    xn[i][:, 128 * j : 128 * j + 128],
                                    eye_r[:],
                                )
                            # dest: 4 j-blocks at cols j*512 + c*128
                            dst = (
                                xtg[i // 4][:]
                                .rearrange("p (j t) -> p j t", t=512)[
                                    :, 4 * jq : 4 * jq + 4,
                                    128 * c : 128 * c + 128,
                                ]
                            )
                            nc.vector.tensor_copy(
                                dst, pt[:].rearrange("p (j t) -> p j t", t=128)
                            )
                        yield unit

            def qkvn_units(g):
                """qk-projection and v-projection for group g."""
                def qk_unit(jpair):
                    def unit():
                        if jpair == 0:
                            qkvn_units.pqk = pQK.tile([128, 512], F32, tag="pqk")
                        pqk = qkvn_units.pqk
                        for j in (2 * jpair, 2 * jpair + 1):
                            nc.tensor.matmul(
                                pqk[:],
                                wqk[:, 128 * j : 128 * j + 128],
                                xtg[g][:, 512 * j : 512 * j + 512],
                                start=(j == 0),
                                stop=(j == NE - 1),
                            )
                        if jpair == 3:
                            nc.vector.tensor_copy(
                                qTt[:, 512 * g : 512 * g + 512], pqk[0:64, :]
                            )
                            nc.scalar.copy(
                                kTt[:, 512 * g : 512 * g + 512], pqk[64:128, :]
                            )
                    return unit
                for jp in range(4):
                    yield qk_unit(jp)

                def v_unit(c):
                    def unit():
                        if c == 0:
                            qkvn_units.pv = pV.tile([128, 256], F32, tag="pv")
                        pv = qkvn_units.pv
                        for j in range(NE):
                            nc.tensor.matmul(
                                pv[:, 64 * c : 64 * c + 64],
                                xtg[g][:, 512 * j + 128 * c : 512 * j + 128 * c + 128],
                                wvb[:, 64 * j : 64 * j + 64],
                                start=(j == 0),
                                stop=(j == NE - 1),
                            )
                        if c == 3:
                            i0 = 4 * g
                            dst = (
                                v1[:]
                                .rearrange("p (i c) -> p i c", c=65)[
                                    :, i0 : i0 + 4, 0:64
                                ]
                            )
                            nc.scalar.copy(
                                dst, pv[:].rearrange("p (i c) -> p i c", c=64)
                            )
                    return unit
                for c in range(4):
                    yield v_unit(c)

            def s_units(g):
                """scoresT + exp for all s-chunks j <= 4g+3 over t-block g."""
                for j in range(4 * g + 4):
                    def unit(j=j, g=g):
                        off = max(0, 128 * j - 512 * g)  # diagonal trim
                        ps = pS.tile([128, 512], F32, tag="ps")
                        o = off
                        while o < 512:
                            n = min(512 - o, 512)
                            nc.tensor.matmul(
                                ps[:, o : o + n],
                                kTt[:, 128 * j : 128 * j + 128],
                                qTt[:, 512 * g + o : 512 * g + o + n],
                                start=True,
                                stop=True,
                            )
                            o += n
                        nc.scalar.activation(
                            wT[j][:, 512 * g + off : 512 * g + 512],
                            ps[:, off:512],
                            mybir.ActivationFunctionType.Exp,
                            scale=SCALE,
                        )
                        if 4 * g <= j:  # diagonal block: mask after exp
                            nc.vector.tensor_mul(
                                wT[j][:, 128 * j : 128 * j + 128],
                                wT[j][:, 128 * j : 128 * j + 128],
                                tri[:],
                            )
                    yield unit

            def av_units(g):
                """AV accumulation for the 4 t-chunks of group g (j-major:
                consecutive matmuls hit different psum col-slices so the
                accumulation chains pipeline) + normalization."""
                def alloc():
                    av_units.pav = pAV.tile([128, 260], F32, tag="pav")
                yield alloc
                for c in range(4):
                    i = 4 * g + c
                    js = list(range(i + 1))
                    batches = [js[k : k + 4] for k in range(0, len(js), 4)]
                    for bi, batch in enumerate(batches):
                        def unit(i=i, c=c, g=g, batch=batch,
                                 last=(bi == len(batches) - 1)):
                            pav = av_units.pav
                            for j in batch:
                                nc.tensor.matmul(
                                    pav[:, 65 * c : 65 * c + 65],
                                    wT[j][:, 128 * i : 128 * i + 128],
                                    v1[:, 65 * j : 65 * j + 65],
                                    start=(j == 0),
                                    stop=(j == i),
                                )
                            if last:
                                rcp = finpool.tile([128, 1], F32, tag="rcp",
                                                   bufs=4)
                                nc.vector.reciprocal(
                                    rcp[:], pav[:, 65 * c + 64 : 65 * c + 65]
                                )
                                nc.vector.tensor_scalar_mul(
                                    stage[g][:, 64 * c : 64 * c + 64],
                                    pav[:, 65 * c : 65 * c + 64],
                                    rcp[:],
                                )
                        yield unit
                def dma_out(g=g):
                    nc.sync.dma_start(
                        out_ap[512 * g : 512 * g + 512, :]
                        .rearrange("(c p) h -> p c h", p=128),
                        stage[g][:].rearrange("p (c h) -> p c h", h=64),
                    )
                yield dma_out

            def drain(*streams):
                streams = [s for s in streams if s is not None]
                while streams:
                    nxt = []
                    for s in streams:
                        u = next(s, None)
                        if u is not None:
                            u()
                            nxt.append(s)
                    streams = nxt

            # ---------- prologue: group 0 transpose + projections ----------
            drain(t_units(0))
            drain(qkvn_units(0))

            # ---------- steady loop ----------
            for g in range(NG):
                a = s_units(g)
                b = av_units(g - 1) if g >= 1 else None
                c = t_units(g + 1) if g + 1 < NG else None
                d = qkvn_units(g + 1) if g + 1 < NG else None
                # round-robin a/b with double-weight c; d after c drains
                streams = [s for s in (a, b) if s is not None]
                cd = c
                c_done = False
                while streams or cd is not None:
                    nxt = []
                    for s in streams:
                        u = next(s, None)
                        if u is not None:
                            u()
                            nxt.append(s)
                            if cd is None and s is a:
                                u = next(s, None)
                                if u is not None:
                                    u()
                    if cd is not None:
                        for _ in range(2):
                            u = next(cd, None)
                            if u is None:
                                if not c_done:
                                    c_done = True
                                    cd = d
                                else:
                                    cd = None
                                break
                            u()
                    streams = nxt

            # ---------- epilogue: AV of the last group ----------
            drain(av_units(NG - 1))

    if split_waits:
        _split_excess_waits(nc)
    return nc


_NC_CACHE = None


def _get_nc() -> bass.Bass:
    global _NC_CACHE
    if _NC_CACHE is None:
        _NC_CACHE = build_nc()
    return _NC_CACHE


def kernel(x, Wq, Wk, Wv, **run_kwargs):
    nc = _get_nc()
    x = np.ascontiguousarray(x, dtype=np.float32)
    in_maps = [
        {
            "x": np.ascontiguousarray(x[b]),
            "Wq": np.ascontiguousarray(Wq, dtype=np.float32),
            "Wk": np.ascontiguousarray(Wk, dtype=np.float32),
            "Wv": np.ascontiguousarray(Wv, dtype=np.float32),
        }
        for b in range(B)
    ]
    res = run_bass_kernel_spmd(nc, in_maps, core_ids=list(range(B)), **run_kwargs)
    out = np.stack([res.results[b]["out"] for b in range(B)], axis=0)
    kernel.last_results = res
    return out
